# revision 9
# baseline (speedup 1.0000x reference)
"""Causal average pooling (AverageContext) Trainium2 Bass kernel.

out[b, t, c] = mean_{s<=t} x[b, s, c]  for x of shape [16, 4096, 128] fp32.

Sharding: data-parallel over batch B=16 across 8 NeuronCores (2 batches/core).

Per-core algorithm (per batch, T=4096 split into 32 tiles of 128):
  SBUF layout x_buf[p, (i, c)] = x[i*128 + p, c]   (t-within-tile on partitions)
  1. Within-tile inclusive cumsum over p: matmul with upper-triangular-ones
     stationary (out = triu.T @ x = tril @ x), 8 matmuls of N=512 (float32r).
  2. Per-tile column sums S[c, i] via 32 small matmuls (lhsT = x_tile, so the
     output lands with c on partitions, tile index on free).
  3. Inclusive prefix over tiles: one vector-engine tensor_tensor_scan.
  4. PE transpose -> [i, c] rows, then SBUF->SBUF DMA to flatten rows into a
     single-partition [1, 4096] exclusive-prefix vector R_flat.
  5. Broadcast-add R_flat onto the cumsum PSUM: K=1 matmuls (lhsT = ones[1,128]),
     N=512 each.
  6. Scale by 1/(t+1): per-partition scalars (inv[p, i]), PSUM->SBUF copies
     alternated between Vector and Scalar engines; DMA out.
"""

import os
import sys

import numpy as np

for _p in (
    "/opt/trn_rl_repo",
    "/root/.axon_site",
    "/root/.axon_site/_ro/trn_rl_repo",
    "/root/.axon_site/_ro/pypackages",
):
    if os.path.isdir(_p) and _p not in sys.path:
        sys.path.append(_p)

import concourse.bass as bass  # noqa: E402
import concourse.mybir as mybir  # noqa: E402
import concourse.tile as tile  # noqa: E402

B, T, C = 16, 4096, 128
NCORES = 8
BPC = B // NCORES  # batches per core
P = 128  # partitions / t-tile size
NTILES = T // P  # 32
NCHUNK = T // 512  # 8 chunks of 512 (4 tiles) each
NQUART = 4  # input DMA split: 4 loads of [128, 1024]

F32 = mybir.dt.float32
F32R = mybir.dt.float32r


def _legalize_sync_waits(nc):
    """Walrus on this stack rejects instructions with more than one sync wait
    ("Too many sync wait commands"), and the fp32/f32r self-loading matmul
    rejects even one (the waits land on the LW slot). Move excess waits onto
    standalone InstEventSemaphore instructions inserted immediately before the
    owning instruction on the same engine.
    """
    uid = [0]

    def mk_wait(engine, w):
        uid[0] += 1
        return mybir.InstEventSemaphore(
            name=f"I-waitfix-{uid[0]}",
            engine=engine,
            ins=[],
            outs=[],
            sync_info=mybir.SyncInfo(on_wait=[w], on_update=[]),
        )

    for f in nc.m.functions:
        for blk in f.blocks:
            out = []
            for inst in blk.instructions:
                si = inst.sync_info
                waits = list(si.on_wait) if si is not None and si.on_wait else []
                keep = 0 if type(inst).__name__ in ("InstMatmult", "InstLdweights") else 1
                if len(waits) > keep:
                    moved = waits[: len(waits) - keep] if keep else waits
                    kept = waits[len(waits) - keep :] if keep else []
                    for w in moved:
                        out.append(mk_wait(inst.engine, w))
                    inst.sync_info = mybir.SyncInfo(
                        on_wait=kept, on_update=list(si.on_update) if si.on_update else []
                    )
                out.append(inst)
            blk.instructions = out


def _build_nc(legalize=True, reps=1):
    from contextlib import ExitStack

    nc = bass.Bass()
    x_in = nc.declare_dram_parameter("x", [BPC, T, C], F32R, isOutput=False)
    triu_in = nc.declare_dram_parameter("triu", [P, P], F32R, isOutput=False)
    ident_in = nc.declare_dram_parameter("ident", [P, P], F32, isOutput=False)
    onesrow_in = nc.declare_dram_parameter("onesrow", [1, P], F32R, isOutput=False)
    onescol_in = nc.declare_dram_parameter("onescol", [P, 2], F32R, isOutput=False)
    inv_in = nc.declare_dram_parameter("invt", [P, NTILES], F32, isOutput=False)
    ones32_in = nc.declare_dram_parameter("ones32", [P, NTILES], F32, isOutput=False)
    y_out = nc.declare_dram_parameter("out", [BPC, T, C], F32, isOutput=True)

    with tile.TileContext(nc) as tc, ExitStack() as ctx:
        consts = ctx.enter_context(tc.tile_pool(name="consts", bufs=1))
        xq_pool = ctx.enter_context(tc.tile_pool(name="xq", bufs=2 * NQUART))
        sb_small = ctx.enter_context(tc.tile_pool(name="sb_small", bufs=2))
        out_pool = ctx.enter_context(tc.tile_pool(name="outp", bufs=4))
        ps_s = ctx.enter_context(tc.tile_pool(name="ps_s", bufs=2, space="PSUM"))
        ps_rt = ctx.enter_context(tc.tile_pool(name="ps_rt", bufs=2, space="PSUM"))
        ps_cs = ctx.enter_context(tc.tile_pool(name="ps_cs", bufs=4, space="PSUM"))

        # constants
        triu = consts.tile([P, P], F32R, tag="triu")
        nc.sync.dma_start(out=triu, in_=triu_in[:, :])
        ident = consts.tile([P, P], F32, tag="ident")
        nc.sync.dma_start(out=ident, in_=ident_in[:, :])
        onesrow = consts.tile([1, P], F32R, tag="onesrow")
        nc.sync.dma_start(out=onesrow, in_=onesrow_in[:, :])
        onescol = consts.tile([P, 2], F32R, tag="onescol")
        nc.sync.dma_start(out=onescol, in_=onescol_in[:, :])
        invt = consts.tile([P, NTILES], F32, tag="invt")
        nc.sync.dma_start(out=invt, in_=inv_in[:, :])
        ones32 = consts.tile([P, NTILES], F32, tag="ones32")
        nc.sync.dma_start(out=ones32, in_=ones32_in[:, :])

        for b in [bb for _ in range(reps) for bb in range(BPC)]:
            x_dram = x_in[b].rearrange("(i p) c -> p i c", p=P)  # [128, 32, 128]
            y_dram = y_out[b].rearrange("(i p) c -> p i c", p=P)

            # load x: 4 DMAs of [128, 8, 128]
            xq = []
            for q in range(NQUART):
                xt = xq_pool.tile([P, NTILES // NQUART, C], F32R, tag="xq")
                nc.sync.dma_start(out=xt, in_=x_dram[:, q * 8 : (q + 1) * 8, :])
                xq.append(xt)

            def xslice(i):
                """x tile i as [128, 128] AP."""
                return xq[i // 8][:, i % 8, :]

            # per-tile column sums -> S_cp[c, i] (c on partitions)
            # f32r matmul needs moving free >= 2: compute each column sum
            # twice (ones[128,2]) and read back every other column.
            s_cp = ps_s.tile([P, 2 * NTILES], F32, tag="s_cp")
            for i in range(NTILES):
                nc.tensor.matmul(
                    s_cp[:, 2 * i : 2 * i + 2], xslice(i), onescol, start=True, stop=True
                )

            s_sb = sb_small.tile([P, NTILES], F32, tag="s_sb")
            nc.vector.tensor_copy(s_sb, s_cp.rearrange("p (i two) -> p i two", two=2)[:, :, 0])
            p_incl = sb_small.tile([P, NTILES], F32, tag="p_incl")
            nc.vector.tensor_tensor_scan(
                p_incl, s_sb, ones32, 0.0, mybir.AluOpType.add, mybir.AluOpType.mult
            )
            # transpose -> [i, c] rows
            rt_ps = ps_rt.tile([NTILES, P], F32, tag="rt_ps")
            nc.tensor.transpose(rt_ps, p_incl, ident)
            rt_sb = sb_small.tile([NTILES, P], F32R, tag="rt_sb")
            nc.vector.tensor_copy(rt_sb, rt_ps)

            # flatten rows 0..30 into [1, 4096] exclusive prefix; the first 128
            # entries (tile 0) are never read, so no zero-fill is needed.
            r_flat = sb_small.tile([1, T], F32R, tag="r_flat")
            nc.gpsimd.dma_start(
                out=r_flat[:, P:T].rearrange("p (i c) -> p i c", c=P),
                in_=rt_sb[0 : NTILES - 1, :],
            )

            for k in range(NCHUNK):
                cs = ps_cs.tile([P, 512], F32, tag="cs")
                nc.tensor.matmul(
                    cs,
                    triu,
                    xq[k // 2][:, (k % 2) * 4 : (k % 2) * 4 + 4, :],
                    start=True,
                    stop=False,
                )
                lo = P if k == 0 else 0  # tile 0 has no prefix to add
                nc.tensor.matmul(
                    cs[:, lo:512],
                    onesrow,
                    r_flat[:, k * 512 + lo : (k + 1) * 512],
                    start=False,
                    stop=True,
                )
                out_t = out_pool.tile([P, 512], F32, tag="out_t")
                for j in range(4):
                    i = 4 * k + j
                    sl = slice(j * P, (j + 1) * P)
                    if j % 2 == 0:
                        nc.vector.tensor_scalar_mul(
                            out_t[:, sl], cs[:, sl], invt[:, i : i + 1]
                        )
                    else:
                        nc.scalar.mul(out_t[:, sl], cs[:, sl], invt[:, i : i + 1])
                nc.sync.dma_start(
                    out=y_dram[:, 4 * k : 4 * k + 4, :],
                    in_=out_t.rearrange("p (i c) -> p i c", c=P),
                )
    if legalize:
        _legalize_sync_waits(nc)
    return nc


def _make_consts():
    triu = np.triu(np.ones((P, P), dtype=np.float32))
    ident = np.eye(P, dtype=np.float32)
    onesrow = np.ones((1, P), dtype=np.float32)
    onescol = np.ones((P, 2), dtype=np.float32)
    t_idx = np.arange(NTILES)[None, :] * P + np.arange(P)[:, None]  # [p, i] -> t
    invt = (1.0 / (t_idx + 1.0)).astype(np.float32)
    ones32 = np.ones((P, NTILES), dtype=np.float32)
    return dict(
        triu=triu, ident=ident, onesrow=onesrow, onescol=onescol,
        invt=invt, ones32=ones32,
    )


_NC = None


def _get_nc():
    global _NC
    if _NC is None:
        _NC = _build_nc()
    return _NC


def kernel(x: np.ndarray) -> np.ndarray:
    from concourse.bass_utils import run_bass_kernel_spmd

    assert x.shape == (B, T, C), x.shape
    nc = _get_nc()
    consts = _make_consts()
    in_maps = [
        {"x": np.ascontiguousarray(x[i * BPC : (i + 1) * BPC]), **consts}
        for i in range(NCORES)
    ]
    res = run_bass_kernel_spmd(nc, in_maps, list(range(NCORES))).results
    return np.concatenate([res[i]["out"] for i in range(NCORES)], axis=0).astype(
        np.float32
    )


if __name__ == "__main__":
    x = np.random.randn(B, T, C).astype(np.float32)
    y = kernel(x)
    ref = np.cumsum(x, axis=1) / (np.arange(T) + 1.0)[None, :, None]
    err = np.abs(y - ref).max() / np.abs(ref).max()
    print("max abs-rel err:", err)


# revision 21
# speedup vs baseline: 24.2473x; 24.2473x over previous
"""Causal average pooling (AverageContext) Trainium2 Bass kernel.

out[b, t, c] = mean_{s<=t} x[b, s, c]  for x [16, 4096, 128] fp32.
Data-parallel over batch: 2 batches per NeuronCore across 8 cores.

Layout (per batch): t = 32*p + j — partition p holds 32 consecutive rows, so
every DMA descriptor moves 16KB contiguous (512B-descriptor patterns are
~10x slower on this stack).

Per batch:
  1. one in-DMA   x3[p, (j, c)] = x[32p + j, c]          SBUF [128, 4096]
  2. GPSIMD halving tree -> G[p, c] = sum_j x3[p, j, c]  (keeps DVE free)
  3. DVE replicates G x4 -> g4 (N=512 matmul moving operand)
  4. PE: 8 matmuls ps = triu_strict.T @ g4 -> B[p, c] = sum_{p'<p} G[p', c]
     (float32r: full-rate for moving dim >= 256), replicated over j
  5. DVE two-level prefix over j on PSUM (j = a*8 + b): per-b steps cover all
     4 a-groups at once, then 3 carry adds -> ps = B + within-partition prefix
  6. ACT scales each j-slice by inv[p, j] = 1/(32p + j + 1) (per-partition
     scalar) into SBUF
  7. one out-DMA (same blocked layout)

A post-pass moves semaphore waits onto standalone InstEventSemaphore
instructions: walrus here rejects >1 wait per instruction and any wait on the
f32r self-loading matmul's LW slot.
"""

import os
import sys

import numpy as np

for _p in (
    "/opt/trn_rl_repo",
    "/root/.axon_site",
    "/root/.axon_site/_ro/trn_rl_repo",
    "/root/.axon_site/_ro/pypackages",
):
    if os.path.isdir(_p) and _p not in sys.path:
        sys.path.append(_p)

import concourse.bass as bass  # noqa: E402
import concourse.mybir as mybir  # noqa: E402
import concourse.tile as tile  # noqa: E402

B, T, C = 16, 4096, 128
NCORES = 8
BPC = B // NCORES
P = 128
J = T // P  # 32 rows per partition
NB = 8  # psum banks
A2, BB = 4, 8  # two-level prefix split: j = a*BB + b

F32 = mybir.dt.float32
F32R = mybir.dt.float32r


def _legalize_sync_waits(nc):
    """Move excess sync waits onto standalone InstEventSemaphore instructions.

    Walrus on this stack rejects instructions with more than one sync wait,
    and the fp32/f32r self-loading matmul rejects even one (waits land on the
    LW slot).
    """
    uid = [0]

    def mk_wait(engine, w):
        uid[0] += 1
        return mybir.InstEventSemaphore(
            name=f"I-waitfix-{uid[0]}",
            engine=engine,
            ins=[],
            outs=[],
            sync_info=mybir.SyncInfo(on_wait=[w], on_update=[]),
        )

    for f in nc.m.functions:
        for blk in f.blocks:
            out = []
            for inst in blk.instructions:
                si = inst.sync_info
                waits = list(si.on_wait) if si is not None and si.on_wait else []
                keep = 0 if type(inst).__name__ in ("InstMatmult", "InstLdweights") else 1
                if len(waits) > keep:
                    moved = waits[: len(waits) - keep] if keep else waits
                    kept = waits[len(waits) - keep :] if keep else []
                    for w in moved:
                        out.append(mk_wait(inst.engine, w))
                    inst.sync_info = mybir.SyncInfo(
                        on_wait=kept,
                        on_update=list(si.on_update) if si.on_update else [],
                    )
                out.append(inst)
            blk.instructions = out


def _build_nc(legalize=True, reps=1):
    from contextlib import ExitStack

    nc = bass.Bass()
    x_in = nc.declare_dram_parameter("x", [BPC, T, C], F32, isOutput=False)
    lstrict_in = nc.declare_dram_parameter("lstrictT", [P, P], F32R, isOutput=False)
    inv_in = nc.declare_dram_parameter("invt", [P, J], F32, isOutput=False)
    y_out = nc.declare_dram_parameter("out", [BPC, T, C], F32, isOutput=True)

    with tile.TileContext(nc) as tc, ExitStack() as ctx:
        consts = ctx.enter_context(tc.tile_pool(name="consts", bufs=1))
        xp = ctx.enter_context(tc.tile_pool(name="xp", bufs=2))
        gp = ctx.enter_context(tc.tile_pool(name="gp", bufs=2))
        op = ctx.enter_context(tc.tile_pool(name="op", bufs=2))
        pp = ctx.enter_context(tc.tile_pool(name="pp", bufs=1, space="PSUM"))

        lstrict = consts.tile([P, P], F32R, tag="lstrict")
        nc.sync.dma_start(out=lstrict, in_=lstrict_in[:, :])
        invt = consts.tile([P, J], F32, tag="invt")
        nc.sync.dma_start(out=invt, in_=inv_in[:, :])

        for r in range(reps):
          for b in range(BPC):
            # rep r>0 re-reads the previous rep's output: keeps repeated
            # bodies live and serially dependent (for reps-diff timing)
            src_t = x_in if r == 0 else y_out
            x_dram = src_t[b].rearrange("(p j) c -> p (j c)", p=P)
            y_dram = y_out[b].rearrange("(p j) c -> p (j c)", p=P)

            x3 = xp.tile([P, J, C], F32, tag="x3")
            nc.sync.dma_start(out=x3, in_=x_dram.rearrange("p (j c) -> p j c", c=C))

            # G = sum_j x3[:, j, :] via halving tree on GPSIMD
            gscr = gp.tile([P, J // 2, C], F32, tag="gscr")
            nc.gpsimd.tensor_add(gscr, x3[:, 0 : J // 2, :], x3[:, J // 2 : J, :])
            w = J // 2
            while w > 1:
                nc.gpsimd.tensor_add(
                    gscr[:, 0 : w // 2, :],
                    gscr[:, 0 : w // 2, :],
                    gscr[:, w // 2 : w, :],
                )
                w //= 2
            g4 = gp.tile([P, 4, C], F32R, tag="g4")
            gb = bass.AP(
                tensor=gscr.tensor,
                offset=gscr.offset,
                ap=[gscr.ap[0], [0, 4], [1, C]],
            )
            nc.vector.tensor_copy(g4, gb)

            # B = exclusive cross-partition prefix of G, replicated over j
            ps = pp.tile([P, J, C], F32, tag="ps")
            # even banks first: the chain's seed/copy ops touch even banks, so
            # they can start after 1-4 matmuls instead of all 8
            for k in (0, 2, 4, 6, 1, 3, 5, 7):
                nc.tensor.matmul(
                    ps[:, 4 * k : 4 * k + 4, :], lstrict, g4, start=True, stop=True
                )

            # two-level within-partition prefix on PSUM
            psr = ps.rearrange("p (a b) c -> p a b c", b=BB)
            x3r = x3.rearrange("p (a b) c -> p a b c", b=BB)
            nc.vector.tensor_add(psr[:, 0, 0, :], psr[:, 0, 0, :], x3r[:, 0, 0, :])
            nc.vector.tensor_copy(psr[:, 1:A2, 0, :], x3r[:, 1:A2, 0, :])
            for bb in range(1, BB):
                nc.vector.tensor_add(
                    psr[:, :, bb, :], psr[:, :, bb - 1, :], x3r[:, :, bb, :]
                )
            carry = gp.tile([P, C], F32, tag="carry")
            for a in range(A2 - 1):
                nc.vector.tensor_copy(carry, psr[:, a, BB - 1, :])
                cb = bass.AP(
                    tensor=carry.tensor,
                    offset=carry.offset,
                    ap=[carry.ap[0], [0, BB], carry.ap[-1]],
                )
                nc.vector.tensor_add(psr[:, a + 1, :, :], psr[:, a + 1, :, :], cb)

            # scale by inv[p, j] on ACT, then one out-DMA. Emitted in
            # reverse j order: the first ACT op waits once for the finished
            # DVE chain and every later wait is pre-satisfied (emitting in
            # chain order makes ACT ratchet ~12 separate waits against chain
            # progress, each costing poll latency on this stack).
            out_t = op.tile([P, J, C], F32, tag="out_t")
            # low half (groups 0-1, final after carry-1) first so it overlaps
            # the rest of the chain; each half reversed so only its first op
            # blocks (2 blocking waits total instead of ~12 ratcheting ones)
            for j in [*reversed(range(J // 2)), *reversed(range(J // 2, J))]:
                nc.scalar.mul(out_t[:, j, :], ps[:, j, :], invt[:, j : j + 1])

            nc.sync.dma_start(out=y_dram, in_=out_t.rearrange("p j c -> p (j c)"))

    if legalize:
        _legalize_sync_waits(nc)
    return nc


def _make_consts():
    lstrictT = np.triu(np.ones((P, P), dtype=np.float32), 1)
    t_idx = np.arange(P)[:, None] * J + np.arange(J)[None, :]
    invt = (1.0 / (t_idx + 1.0)).astype(np.float32)
    return dict(lstrictT=lstrictT, invt=invt)


_NC = None


def _get_nc():
    global _NC
    if _NC is None:
        _NC = _build_nc()
    return _NC


def kernel(x: np.ndarray) -> np.ndarray:
    from concourse.bass_utils import run_bass_kernel_spmd

    assert x.shape == (B, T, C), x.shape
    x = np.asarray(x, dtype=np.float32)
    nc = _get_nc()
    consts = _make_consts()
    in_maps = [
        {"x": np.ascontiguousarray(x[i * BPC : (i + 1) * BPC]), **consts}
        for i in range(NCORES)
    ]
    res = run_bass_kernel_spmd(nc, in_maps, list(range(NCORES))).results
    return np.concatenate([res[i]["out"] for i in range(NCORES)], axis=0).astype(
        np.float32
    )


if __name__ == "__main__":
    x = np.random.randn(B, T, C).astype(np.float32)
    y = kernel(x)
    ref = np.cumsum(x, axis=1) / (np.arange(T) + 1.0)[None, :, None]
    err = np.abs(y - ref).max() / np.abs(ref).max()
    print("max abs-rel err:", err)
